# revision 1
# baseline (speedup 1.0000x reference)
"""LoRA Linear (x @ W.T + b + scaling * (x @ A.T) @ B.T) on 8 TRN2 NeuronCores.

Strategy:
  - Fold the LoRA adapter into the dense weight on host:
        Wf = W + (alpha/rank) * (lora_B @ lora_A)        (exact algebra)
    so the device kernel is a single dense matmul + bias.
  - Data-parallel: shard the 8192 tokens into 8 x 1024 rows, one shard per core.
    W is replicated (sharding_hint).
  - Per core: out[m, o] = sum_k xT[k, m] * WT[k, o] + b[o]
    PE matmul tiles: lhsT = xT[k128, m128] (stationary), rhs = WT[k128, o512]
    (moving), accumulate 32 k-tiles into a [128, 512] PSUM bank, DVE adds the
    (pre-broadcast) bias while copying PSUM -> SBUF, DMA to out.
  - bf16 matmul operands (PE runs bf16 at full rate, fp32 at 1/4 rate),
    fp32 PSUM accumulation.
"""

import numpy as np
import ml_dtypes

import concourse.bass as bass
from concourse import bacc
import concourse.mybir as mybir
import concourse.tile as tile
from concourse.bass_utils import run_bass_kernel_spmd

N_CORES = 8
IN_F = 4096
OUT_F = 4096
RANK = 16
ALPHA = 32.0
B_SZ = 4
S_SZ = 2048
TOK = B_SZ * S_SZ          # 8192
M_PER_CORE = TOK // N_CORES  # 1024

P = 128                    # partitions
KT = IN_F // P             # 32 k-tiles
O_BLK = 512                # o-block width (matmul moving free dim)
N_OBLK = OUT_F // O_BLK    # 8
MT = M_PER_CORE // P       # 8 m-tiles

MM_DT = mybir.dt.bfloat16
NP_MM_DT = ml_dtypes.bfloat16

LAST_RESULTS = None        # test.py reads exec_time_ns from here


KTB = KT + 1               # extra k-tile carries the bias row


def _build_nc(trace_scopes=False):
    nc = bacc.Bacc(None, target_bir_lowering=False)

    xt_d = nc.dram_tensor("xt", [KTB, P, M_PER_CORE], MM_DT, kind="ExternalInput")
    wt_d = nc.dram_tensor("wt", [N_OBLK, KTB, P, O_BLK], MM_DT, kind="ExternalInput")
    out_d = nc.dram_tensor("out", [M_PER_CORE, OUT_F], mybir.dt.float32,
                           kind="ExternalOutput")

    with tile.TileContext(nc) as tc:
        with (
            tc.tile_pool(name="xt", bufs=1) as xt_pool,
            tc.tile_pool(name="wt", bufs=2) as wt_pool,
            tc.tile_pool(name="outs", bufs=8) as out_pool,
            tc.tile_pool(name="psum", bufs=8, space="PSUM") as psum_pool,
        ):
            # Per-k-tile loads (instead of one monolithic DMA) so the first
            # matmuls only wait on their own k-slice: ~18 us faster startup.
            xts = []
            for k in range(KTB):
                t = xt_pool.tile([P, M_PER_CORE], MM_DT, tag=f"xt{k}")
                nc.sync.dma_start(t[:], xt_d[k])
                xts.append(t)

            for ob in range(N_OBLK):
                wts = []
                for k in range(KTB):
                    t = wt_pool.tile([P, O_BLK], MM_DT, tag=f"wt{k}")
                    nc.sync.dma_start(t[:], wt_d[ob, k])
                    wts.append(t)

                for mt in range(MT):
                    psum = psum_pool.tile([P, O_BLK], mybir.dt.float32)
                    for k in range(KTB):
                        nc.tensor.matmul(
                            psum[:],
                            xts[k][:, mt * P:(mt + 1) * P],
                            wts[k][:],
                            start=(k == 0),
                            stop=(k == KTB - 1),
                        )
                    out_sb = out_pool.tile([P, O_BLK], mybir.dt.float32)
                    nc.any.tensor_copy(out=out_sb[:], in_=psum[:])
                    nc.sync.dma_start(
                        out_d[mt * P:(mt + 1) * P, ob * O_BLK:(ob + 1) * O_BLK],
                        out_sb[:],
                    )
    nc.compile()
    return nc


_NC_CACHE = None


def kernel(x, W, b, lora_A, lora_B, _trace=False):
    global LAST_RESULTS, _NC_CACHE

    # ---- host prep ----
    scaling = ALPHA / RANK
    Wf = (W.astype(np.float64)
          + scaling * (lora_B.astype(np.float64) @ lora_A.astype(np.float64)))
    # WT[k, o] = Wf[o, k]; pre-tiled into o-blocks: [N_OBLK, KT, P, O_BLK],
    # plus one extra k-tile whose partition-0 row carries the bias.
    WT = np.ascontiguousarray(Wf.T).astype(NP_MM_DT)          # [IN_F, OUT_F]
    wt_in = np.zeros((N_OBLK, KTB, P, O_BLK), dtype=NP_MM_DT)
    wt_in[:, :KT] = WT.reshape(KT, P, N_OBLK, O_BLK).transpose(2, 0, 1, 3)
    b_blk = b.astype(np.float32).reshape(N_OBLK, O_BLK)
    wt_in[:, KT, 0, :] = b_blk.astype(NP_MM_DT)

    x_flat = np.ascontiguousarray(x.reshape(TOK, IN_F))

    in_maps = []
    for c in range(N_CORES):
        xc = x_flat[c * M_PER_CORE:(c + 1) * M_PER_CORE]       # [1024, 4096]
        xt = np.ascontiguousarray(xc.T).astype(NP_MM_DT)       # [4096, 1024]
        xt_in = np.zeros((KTB, P, M_PER_CORE), dtype=NP_MM_DT)
        xt_in[:KT] = xt.reshape(KT, P, M_PER_CORE)
        xt_in[KT, 0, :] = 1.0                                  # bias one-hot row
        in_maps.append({
            "xt": xt_in,
            "wt": wt_in,
        })

    if _NC_CACHE is None:
        _NC_CACHE = _build_nc()
    nc = _NC_CACHE

    res = run_bass_kernel_spmd(nc, in_maps, core_ids=list(range(N_CORES)),
                               trace=_trace)
    LAST_RESULTS = res

    out = np.concatenate([r["out"] for r in res.results], axis=0)
    return out.reshape(B_SZ, S_SZ, OUT_F).astype(np.float32)



# revision 3
# speedup vs baseline: 2.6037x; 2.6037x over previous
"""LoRA Linear (x @ W.T + b + scaling * (x @ A.T) @ B.T) on 8 TRN2 NeuronCores.

Strategy (v2, fp8 DoubleRow):
  - Data-parallel: 8192 tokens -> 8 x 1024 rows, one shard per core.
  - Base matmul in fp8 e4m3 with MatmulPerfMode.DoubleRow (2 k-rows per
    partition per instruction, 0.5 cycles per output row): ~4x the bf16
    matmul rate under the instruction cost model.
  - Precision: fp8 quantization noise of x and W alone gives ~1.4% rel err,
    but ONLY if the LoRA adapter is NOT folded into W (folded-fp8 is ~3.9%:
    the rank-16 adapter dominates the output and would amplify both x's and
    W's quantization error). So the adapter path runs in bf16 on device:
        xa = x_bf16 @ A.T           (PE, tiny moving free dim -> ~free)
        xaT = transpose(xa)         (PE transpose via identity)
        adj = xaT.T @ [2*B.T; b]    (one bf16 matmul closing each PSUM
                                     accumulation group; row 16 of xaT is
                                     ones so the bias rides along)
  - Scales: x*32 and W*2048 keep fp8 values out of the subnormal range
    (max|x*32| ~ 173, max|W*2048| ~ 173 < 240). PSUM therefore holds
    65536*out; host divides by 2^16 after gather (exact, power of two).
  - Output returned as bf16 (halves the out DMA), cast to fp32 on host.
  - Schedule: x m-tiles and W o-blocks stream in interleaved; PE processes
    o-blocks 0,1 as an interleaved 2-column band while x is still loading,
    then o-blocks 2..7 column-major. W tiles triple-buffered; W o-blocks
    3..7 are enqueued after all x DMAs (FIFO queue: a slot-gated W DMA
    ahead of x DMAs would deadlock against the groups that must consume
    that x to free the slot).
"""

import numpy as np
import ml_dtypes

import concourse.bass as bass
from concourse import bacc
import concourse.mybir as mybir
import concourse.tile as tile
from concourse.bass_utils import run_bass_kernel_spmd

N_CORES = 8
IN_F = 4096
OUT_F = 4096
RANK = 16
ALPHA = 32.0
B_SZ = 4
S_SZ = 2048
TOK = B_SZ * S_SZ            # 8192
M_PER_CORE = TOK // N_CORES  # 1024

P = 128                      # partitions
KT = IN_F // P               # 32 k-tiles
KP = KT // 2                 # 16 k-pair tiles (DoubleRow)
O_BLK = 512                  # o-block width (psum bank = 512 fp32)
N_OBLK = OUT_F // O_BLK      # 8
MT = M_PER_CORE // P         # 8 m-tiles
RP = RANK + 1                # rank rows + ones row (bias)

SX = 32.0                    # x fp8 scale
SW = 2048.0                  # W fp8 scale
DESCALE = 1.0 / (SX * SW)    # applied on host (power of two, exact)

F8 = mybir.dt.float8e4
BF = mybir.dt.bfloat16
NP_F8 = ml_dtypes.float8_e4m3
NP_BF = ml_dtypes.bfloat16

LAST_RESULTS = None          # test.py reads exec_time_ns from here
_NC_CACHE = None


def _build_nc():
    nc = bacc.Bacc(None, target_bir_lowering=False)

    xf8_d = nc.dram_tensor("xf8", [MT, P, KT, P], F8, kind="ExternalInput")
    xbf_d = nc.dram_tensor("xbf", [MT, P, KT, P], BF, kind="ExternalInput")
    wt_d = nc.dram_tensor("wt", [N_OBLK, KP, P, 2, O_BLK], F8, kind="ExternalInput")
    at_d = nc.dram_tensor("at", [P, KT, RANK], BF, kind="ExternalInput")
    bb_d = nc.dram_tensor("bb", [RP, OUT_F], BF, kind="ExternalInput")
    id_d = nc.dram_tensor("ident", [P, P], BF, kind="ExternalInput")
    out_d = nc.dram_tensor("out", [M_PER_CORE, OUT_F], BF, kind="ExternalOutput")

    with tile.TileContext(nc) as tc:
        with (
            tc.tile_pool(name="xp", bufs=1) as xp,
            tc.tile_pool(name="wp", bufs=3) as wp,
            tc.tile_pool(name="sm", bufs=1) as sm,
            tc.tile_pool(name="outs", bufs=8) as outs,
            tc.tile_pool(name="pmain", bufs=5, space="PSUM") as pmain,
            tc.tile_pool(name="pxa", bufs=2, space="PSUM") as pxa,
            tc.tile_pool(name="ptr", bufs=1, space="PSUM") as ptr,
        ):
            # ---- small constants ----
            at_sb = sm.tile([P, KT, RANK], BF, tag="at")
            nc.sync.dma_start(at_sb[:], at_d[:])
            bb_sb = sm.tile([RP, OUT_F], BF, tag="bb")
            nc.sync.dma_start(bb_sb[:], bb_d[:])
            id_sb = sm.tile([P, P], BF, tag="ident")
            nc.sync.dma_start(id_sb[:], id_d[:])
            xaT = sm.tile([RP, M_PER_CORE], BF, tag="xaT")
            nc.vector.memset(xaT[RANK:RP, :], 1.0)

            xf8s, xbfs = [], []
            for i in range(MT):
                xf8s.append(xp.tile([P, KT, P], F8, tag=f"xf8_{i}", name=f"xf8_{i}"))
                xbfs.append(xp.tile([P, KT, P], BF, tag=f"xbf_{i}", name=f"xbf_{i}"))

            w_tiles = {}

            def dma_x(i):
                nc.sync.dma_start(xf8s[i][:], xf8_d[i])
                nc.sync.dma_start(xbfs[i][:], xbf_d[i])

            def dma_w(j):
                tiles = []
                for kp in range(KP):
                    t = wp.tile([P, 2, O_BLK], F8, tag=f"w{kp}", name=f"w{j}_{kp}")
                    nc.sync.dma_start(t[:], wt_d[j, kp])
                    tiles.append(t)
                w_tiles[j] = tiles

            def xa(i):
                # xa_psum[tok, r] = sum_k x_bf16[tok, k] * A[r, k]
                ps = pxa.tile([P, RANK], mybir.dt.float32, tag="pxa")
                for k in range(KT):
                    nc.tensor.matmul(
                        ps[:], xbfs[i][:, k, :], at_sb[:, k, :],
                        start=(k == 0), stop=(k == KT - 1),
                    )
                xa_sb = sm.tile([P, RANK], BF, tag="xa_sb", bufs=2)
                nc.scalar.copy(xa_sb[:], ps[:])
                tr = ptr.tile([RANK, P], BF, tag="ptr")
                nc.tensor.transpose(tr[:], xa_sb[:], id_sb[:])
                nc.scalar.copy(xaT[0:RANK, i * P:(i + 1) * P], tr[:])

            gidx = 0

            def group(i, j):
                nonlocal gidx
                ps = pmain.tile([P, O_BLK], mybir.dt.float32, tag="pm")
                for kp in range(KP):
                    nc.tensor.matmul(
                        ps[:], xf8s[i][:, 2 * kp:2 * kp + 2, :],
                        w_tiles[j][kp][:],
                        start=(kp == 0), stop=False,
                        perf_mode=mybir.MatmulPerfMode.DoubleRow,
                    )
                nc.tensor.matmul(
                    ps[:], xaT[:, i * P:(i + 1) * P],
                    bb_sb[:, j * O_BLK:(j + 1) * O_BLK],
                    start=False, stop=True,
                )
                o_sb = outs.tile([P, O_BLK], BF, tag="osb")
                if gidx % 2 == 0:
                    nc.scalar.copy(o_sb[:], ps[:])
                else:
                    nc.vector.tensor_copy(o_sb[:], ps[:])
                gidx += 1
                nc.sync.dma_start(
                    out_d[i * P:(i + 1) * P, j * O_BLK:(j + 1) * O_BLK], o_sb[:],
                )

            # ---- DMA + PE emission, interleaved ----
            # W o-blocks 0..2 fill the 3 buffer slots ungated; 3..7 are
            # slot-gated and MUST come after every x DMA (FIFO deadlock).
            dma_x(0)
            dma_w(0)
            dma_x(1)
            xa(0)
            dma_w(1)
            dma_x(2)
            xa(1)
            group(0, 0)
            dma_w(2)
            dma_x(3)
            xa(2)
            group(1, 0)
            group(0, 1)
            dma_x(4)
            xa(3)
            group(2, 0)
            group(1, 1)
            dma_x(5)
            xa(4)
            group(3, 0)
            group(2, 1)
            dma_x(6)
            xa(5)
            group(4, 0)
            group(3, 1)
            dma_x(7)
            xa(6)
            group(5, 0)
            group(4, 1)
            xa(7)
            group(6, 0)
            group(5, 1)
            group(7, 0)
            group(6, 1)
            group(7, 1)
            for j in range(3, N_OBLK):
                dma_w(j)
            for j in range(2, N_OBLK):
                for i in range(MT):
                    group(i, j)
    nc.compile()
    return nc


def _prep_inputs(x, W, b, lora_A, lora_B):
    scaling = ALPHA / RANK
    Wq = (W.astype(np.float32) * SW).astype(NP_F8)            # [out, in]
    # wt[ob, kp, p, t, c] = Wq[ob*512 + c, (2*kp + t)*128 + p]
    wt_in = np.ascontiguousarray(
        Wq.T.reshape(KP, 2, P, N_OBLK, O_BLK).transpose(3, 0, 2, 1, 4)
    )

    at_in = np.ascontiguousarray(
        lora_A.astype(np.float32).T.reshape(KT, P, RANK).transpose(1, 0, 2)
    ).astype(NP_BF)                                           # [p, k, r]

    bb_in = np.zeros((RP, OUT_F), dtype=NP_BF)
    bb_in[0:RANK] = (lora_B.astype(np.float32).T * (scaling * SX * SW)).astype(NP_BF)
    bb_in[RANK] = (b.astype(np.float32) * (SX * SW)).astype(NP_BF)

    id_in = np.eye(P, dtype=NP_BF)

    x_flat = np.ascontiguousarray(x.reshape(TOK, IN_F).astype(np.float32))
    in_maps = []
    for c in range(N_CORES):
        xc = x_flat[c * M_PER_CORE:(c + 1) * M_PER_CORE]      # [1024, 4096]
        # x[m, p, k, c] = xc[m*128 + c, k*128 + p]
        xt = xc.T.reshape(KT, P, MT, P).transpose(2, 1, 0, 3)
        xf8_in = np.ascontiguousarray((xt * SX)).astype(NP_F8)
        xbf_in = np.ascontiguousarray(xt).astype(NP_BF)
        in_maps.append({
            "xf8": xf8_in,
            "xbf": xbf_in,
            "wt": wt_in,
            "at": at_in,
            "bb": bb_in,
            "ident": id_in,
        })
    return in_maps


def kernel(x, W, b, lora_A, lora_B, _trace=False):
    global LAST_RESULTS, _NC_CACHE

    in_maps = _prep_inputs(x, W, b, lora_A, lora_B)

    if _NC_CACHE is None:
        _NC_CACHE = _build_nc()
    nc = _NC_CACHE

    res = run_bass_kernel_spmd(nc, in_maps, core_ids=list(range(N_CORES)),
                               trace=_trace)
    LAST_RESULTS = res

    out = np.concatenate([r["out"].astype(np.float32) for r in res.results], axis=0)
    out *= np.float32(DESCALE)
    return out.reshape(B_SZ, S_SZ, OUT_F).astype(np.float32)


# revision 4
# speedup vs baseline: 3.0399x; 1.1676x over previous
"""LoRA Linear (x @ W.T + b + scaling * (x @ A.T) @ B.T) on 8 TRN2 NeuronCores.

Strategy (fp8 DoubleRow):
  - Data-parallel: 8192 tokens -> 8 x 1024 rows, one shard per core.
  - Base matmul in fp8 e4m3 with MatmulPerfMode.DoubleRow (2 k-rows per
    partition per instruction, 0.5 cycles per output row).
  - Precision: fp8 quantization noise of x and W alone gives ~1.4% rel err,
    but ONLY if the LoRA adapter is NOT folded into W (folded-fp8 is ~3.9%:
    the rank-16 adapter dominates the output and would amplify both x's and
    W's quantization error). So the adapter path runs in bf16 on device:
        xa = x_bf16 @ A.T           (PE, 16-wide moving free dim -> ~free)
        xaT = transpose(xa)         (PE transpose via identity)
        adj = xaT.T @ [2*B.T; b]    (one bf16 matmul closing each PSUM
                                     accumulation group; row 16 of xaT is
                                     ones so the bias rides along)
  - Scales: x*32 and W*2048 keep fp8 values out of the subnormal range
    (max ~173 < 240). PSUM holds 65536*out; host divides by 2^16 (exact).
  - Output bf16 (halves out DMA), fp32 conversion on host.
  - DMA: few large transfers (per-DMA HWDGE prep is ~625ns exclusive):
    one DMA per W o-block, one per x m-tile per dtype, one per output
    column (column-staging SBUF tile). Single SP queue; slot-gated W
    DMAs (o-blocks 3..7, triple-buffered tag) are enqueued only after
    everything needed to free their slot (FIFO deadlock otherwise).
  - PE order: o-blocks 0,1 interleaved as a 2-column band while x still
    streams in, then o-blocks 2..7 column-major.
"""

import numpy as np
import ml_dtypes

import concourse.bass as bass
from concourse import bacc
import concourse.mybir as mybir
import concourse.tile as tile
from concourse.bass_utils import run_bass_kernel_spmd

N_CORES = 8
IN_F = 4096
OUT_F = 4096
RANK = 16
ALPHA = 32.0
B_SZ = 4
S_SZ = 2048
TOK = B_SZ * S_SZ            # 8192
M_PER_CORE = TOK // N_CORES  # 1024

P = 128                      # partitions
KT = IN_F // P               # 32 k-tiles
KP = KT // 2                 # 16 k-pair tiles (DoubleRow)
O_BLK = 512                  # o-block width (psum bank = 512 fp32)
N_OBLK = OUT_F // O_BLK      # 8
MT = M_PER_CORE // P         # 8 m-tiles
RP = RANK + 1                # rank rows + ones row (bias)

SX = 32.0                    # x fp8 scale
SW = 2048.0                  # W fp8 scale
DESCALE = 1.0 / (SX * SW)    # applied on host (power of two, exact)

F8 = mybir.dt.float8e4
BF = mybir.dt.bfloat16
NP_F8 = ml_dtypes.float8_e4m3
NP_BF = ml_dtypes.bfloat16

LAST_RESULTS = None          # test.py reads exec_time_ns from here
_NC_CACHE = None


def _default_schedule():
    # Pre-loop DMA emission order: x m-tile loads and ungated W o-blocks.
    dma_seq = ["x0", "w0", "x1", "x2", "w1", "x3", "x4", "x5", "x6", "x7",
               "w2"]
    # PE emission order: ("xa", i) adapter projections and ("g", i, j)
    # psum groups. 2-column band over o-blocks 0,1 while x streams in.
    pe_seq = [("xa", 0), ("xa", 1), ("g", 0, 0), ("xa", 2), ("g", 1, 0),
              ("g", 0, 1), ("xa", 3), ("g", 2, 0), ("g", 1, 1), ("xa", 4),
              ("g", 3, 0), ("g", 2, 1), ("xa", 5), ("g", 4, 0), ("g", 3, 1),
              ("xa", 6), ("g", 5, 0), ("g", 4, 1), ("xa", 7), ("g", 6, 0),
              ("g", 5, 1), ("g", 7, 0), ("g", 6, 1), ("g", 7, 1)]
    for j in range(2, N_OBLK):
        for i in range(MT):
            pe_seq.append(("g", i, j))
    return dma_seq, pe_seq


def _build_nc(schedule=None):
    dma_seq, pe_seq = schedule if schedule is not None else _default_schedule()

    nc = bacc.Bacc(None, target_bir_lowering=False)

    xf8_d = nc.dram_tensor("xf8", [MT, P, KT, P], F8, kind="ExternalInput")
    xbf_d = nc.dram_tensor("xbf", [MT, P, KT, P], BF, kind="ExternalInput")
    wt_d = nc.dram_tensor("wt", [N_OBLK, P, KP, 2, O_BLK], F8,
                          kind="ExternalInput")
    at_d = nc.dram_tensor("at", [P, KT, RANK], BF, kind="ExternalInput")
    bb_d = nc.dram_tensor("bb", [RP, OUT_F], BF, kind="ExternalInput")
    id_d = nc.dram_tensor("ident", [P, P], BF, kind="ExternalInput")
    # [i][p][j][c] row-major == [1024, 4096] row-major
    out_d = nc.dram_tensor("out", [MT, P, N_OBLK, O_BLK], BF,
                           kind="ExternalOutput")

    with tile.TileContext(nc) as tc:
        with (
            tc.tile_pool(name="xp", bufs=1) as xp,
            tc.tile_pool(name="wp", bufs=3) as wp,
            tc.tile_pool(name="sm", bufs=1) as sm,
            tc.tile_pool(name="outs", bufs=2) as outs,
            tc.tile_pool(name="pmain", bufs=5, space="PSUM") as pmain,
            tc.tile_pool(name="pxa", bufs=2, space="PSUM") as pxa,
            tc.tile_pool(name="ptr", bufs=1, space="PSUM") as ptr,
        ):
            # ---- small constants ----
            at_sb = sm.tile([P, KT, RANK], BF, tag="at")
            nc.sync.dma_start(at_sb[:], at_d[:])
            bb_sb = sm.tile([RP, OUT_F], BF, tag="bb")
            nc.sync.dma_start(bb_sb[:], bb_d[:])
            id_sb = sm.tile([P, P], BF, tag="ident")
            nc.sync.dma_start(id_sb[:], id_d[:])
            xaT = sm.tile([RP, M_PER_CORE], BF, tag="xaT")
            nc.vector.memset(xaT[RANK:RP, :], 1.0)

            xf8s, xbfs = [], []
            for i in range(MT):
                xf8s.append(xp.tile([P, KT, P], F8, tag=f"xf8_{i}",
                                    name=f"xf8_{i}"))
                xbfs.append(xp.tile([P, KT, P], BF, tag=f"xbf_{i}",
                                    name=f"xbf_{i}"))

            w_tiles = {}
            col_tiles = {}
            emitted_w = set()

            def dma_x(i):
                nc.sync.dma_start(xf8s[i][:], xf8_d[i])
                nc.sync.dma_start(xbfs[i][:], xbf_d[i])

            def dma_w(j):
                if j in emitted_w or j >= N_OBLK:
                    return
                emitted_w.add(j)
                t = wp.tile([P, KP, 2, O_BLK], F8, tag="w", name=f"w{j}")
                nc.sync.dma_start(t[:], wt_d[j])
                w_tiles[j] = t

            def xa(i):
                # xa_psum[tok, r] = sum_k x_bf16[tok, k] * A[r, k]
                ps = pxa.tile([P, RANK], mybir.dt.float32, tag="pxa",
                              name=f"pxa{i}")
                for k in range(KT):
                    nc.tensor.matmul(
                        ps[:], xbfs[i][:, k, :], at_sb[:, k, :],
                        start=(k == 0), stop=(k == KT - 1),
                    )
                xa_sb = sm.tile([P, RANK], BF, tag="xa_sb", bufs=2,
                                name=f"xa_sb{i}")
                nc.scalar.copy(xa_sb[:], ps[:])
                tr = ptr.tile([RANK, P], BF, tag="ptr", name=f"ptr{i}")
                nc.tensor.transpose(tr[:], xa_sb[:], id_sb[:])
                nc.scalar.copy(xaT[0:RANK, i * P:(i + 1) * P], tr[:])

            gidx = 0
            col_remaining = {j: MT for j in range(N_OBLK)}

            def group(i, j):
                nonlocal gidx
                if j not in col_tiles:
                    col_tiles[j] = outs.tile([P, MT, O_BLK], BF, tag="col",
                                             name=f"col{j}")
                ps = pmain.tile([P, O_BLK], mybir.dt.float32, tag="pm",
                                name=f"pm{i}_{j}")
                for kp in range(KP):
                    nc.tensor.matmul(
                        ps[:], xf8s[i][:, 2 * kp:2 * kp + 2, :],
                        w_tiles[j][:, kp, :, :],
                        start=(kp == 0), stop=False,
                        perf_mode=mybir.MatmulPerfMode.DoubleRow,
                    )
                nc.tensor.matmul(
                    ps[:], xaT[:, i * P:(i + 1) * P],
                    bb_sb[:, j * O_BLK:(j + 1) * O_BLK],
                    start=False, stop=True,
                )
                if gidx % 2 == 0:
                    nc.scalar.copy(col_tiles[j][:, i, :], ps[:])
                else:
                    nc.vector.tensor_copy(col_tiles[j][:, i, :], ps[:])
                gidx += 1
                col_remaining[j] -= 1
                if col_remaining[j] == 0:
                    nc.sync.dma_start(
                        out_d[:, :, j, :].rearrange("i p c -> p i c"),
                        col_tiles[j][:],
                    )
                    dma_w(j + 3)

            for tok in dma_seq:
                if tok[0] == "x":
                    dma_x(int(tok[1:]))
                else:
                    dma_w(int(tok[1:]))
            for op in pe_seq:
                if op[0] == "xa":
                    xa(op[1])
                else:
                    group(op[1], op[2])
    nc.compile()
    return nc


def _prep_inputs(x, W, b, lora_A, lora_B):
    scaling = ALPHA / RANK
    Wq = (W.astype(np.float32) * SW).astype(NP_F8)            # [out, in]
    # wt[ob, p, kp, t, c] = Wq[ob*512 + c, (2*kp + t)*128 + p]
    wt_in = np.ascontiguousarray(
        Wq.T.reshape(KP, 2, P, N_OBLK, O_BLK).transpose(3, 2, 0, 1, 4)
    )

    at_in = np.ascontiguousarray(
        lora_A.astype(np.float32).T.reshape(KT, P, RANK).transpose(1, 0, 2)
    ).astype(NP_BF)                                           # [p, k, r]

    bb_in = np.zeros((RP, OUT_F), dtype=NP_BF)
    bb_in[0:RANK] = (lora_B.astype(np.float32).T * (scaling * SX * SW)).astype(NP_BF)
    bb_in[RANK] = (b.astype(np.float32) * (SX * SW)).astype(NP_BF)

    id_in = np.eye(P, dtype=NP_BF)

    x_flat = np.ascontiguousarray(x.reshape(TOK, IN_F).astype(np.float32))
    in_maps = []
    for c in range(N_CORES):
        xc = x_flat[c * M_PER_CORE:(c + 1) * M_PER_CORE]      # [1024, 4096]
        # x[m, p, k, c] = xc[m*128 + c, k*128 + p]
        xt = xc.T.reshape(KT, P, MT, P).transpose(2, 1, 0, 3)
        xf8_in = np.ascontiguousarray(xt * SX).astype(NP_F8)
        xbf_in = np.ascontiguousarray(xt).astype(NP_BF)
        in_maps.append({
            "xf8": xf8_in,
            "xbf": xbf_in,
            "wt": wt_in,
            "at": at_in,
            "bb": bb_in,
            "ident": id_in,
        })
    return in_maps


def kernel(x, W, b, lora_A, lora_B, _trace=False):
    global LAST_RESULTS, _NC_CACHE

    in_maps = _prep_inputs(x, W, b, lora_A, lora_B)

    if _NC_CACHE is None:
        _NC_CACHE = _build_nc()
    nc = _NC_CACHE

    res = run_bass_kernel_spmd(nc, in_maps, core_ids=list(range(N_CORES)),
                               trace=_trace)
    LAST_RESULTS = res

    out = np.concatenate(
        [r["out"].reshape(M_PER_CORE, OUT_F).astype(np.float32)
         for r in res.results], axis=0)
    out *= np.float32(DESCALE)
    return out.reshape(B_SZ, S_SZ, OUT_F).astype(np.float32)


# revision 6
# speedup vs baseline: 3.0679x; 1.0092x over previous
"""LoRA Linear (x @ W.T + b + scaling * (x @ A.T) @ B.T) on 8 TRN2 NeuronCores.

Strategy (fp8 DoubleRow):
  - Data-parallel: 8192 tokens -> 8 x 1024 rows, one shard per core.
  - Base matmul in fp8 e4m3 with MatmulPerfMode.DoubleRow (2 k-rows per
    partition per instruction, 0.5 cycles per output row).
  - Precision: fp8 quantization noise of x and W alone gives ~1.4% rel err,
    but ONLY if the LoRA adapter is NOT folded into W (folded-fp8 is ~3.9%:
    the rank-16 adapter dominates the output and would amplify both x's and
    W's quantization error). So the adapter path runs in bf16 on device:
        xa = x_bf16 @ A.T           (PE, 16-wide moving free dim -> ~free)
        xaT = transpose(xa)         (PE transpose via identity)
        adj = xaT.T @ [2*B.T; b]    (one bf16 matmul closing each PSUM
                                     accumulation group; row 16 of xaT is
                                     ones so the bias rides along)
  - Scales: x*32 and W*2048 keep fp8 values out of the subnormal range
    (max ~173 < 240). PSUM holds 65536*out; host divides by 2^16 (exact).
  - Output bf16 (halves out DMA), fp32 conversion on host.
  - DMA: few large transfers (per-DMA HWDGE prep is ~625ns exclusive):
    one DMA per W o-block, one per x m-tile per dtype, one per output
    column (column-staging SBUF tile). Single SP queue; slot-gated W
    DMAs (o-blocks 3..7, triple-buffered tag) are enqueued only after
    everything needed to free their slot (FIFO deadlock otherwise).
  - PE order: o-blocks 0,1 interleaved as a 2-column band while x still
    streams in, then o-blocks 2..7 column-major.
"""

import numpy as np
import ml_dtypes

import concourse.bass as bass
from concourse import bacc
import concourse.mybir as mybir
import concourse.tile as tile
from concourse.bass_utils import run_bass_kernel_spmd

N_CORES = 8
IN_F = 4096
OUT_F = 4096
RANK = 16
ALPHA = 32.0
B_SZ = 4
S_SZ = 2048
TOK = B_SZ * S_SZ            # 8192
M_PER_CORE = TOK // N_CORES  # 1024

P = 128                      # partitions
KT = IN_F // P               # 32 k-tiles
KP = KT // 2                 # 16 k-pair tiles (DoubleRow)
O_BLK = 512                  # o-block width (psum bank = 512 fp32)
N_OBLK = OUT_F // O_BLK      # 8
MT = M_PER_CORE // P         # 8 m-tiles
RP = RANK + 1                # rank rows + ones row (bias)

SX = 32.0                    # x fp8 scale
SW = 2048.0                  # W fp8 scale
DESCALE = 1.0 / (SX * SW)    # applied on host (power of two, exact)

F8 = mybir.dt.float8e4
BF = mybir.dt.bfloat16
NP_F8 = ml_dtypes.float8_e4m3
NP_BF = ml_dtypes.bfloat16

LAST_RESULTS = None          # test.py reads exec_time_ns from here
_NC_CACHE = None


def _default_schedule():
    # Pre-loop DMA emission order: x m-tile loads and ungated W o-blocks.
    dma_seq = ["x0", "w0", "x1", "x2", "w1", "x3", "x4", "x5", "x6", "x7",
               "w2"]
    # PE emission order: ("d", n) warmup matmuls (keep the PE p-state ramp
    # warm while the first tiles stream in), ("xa", i) adapter projections,
    # ("g", i, j) psum groups. 2-column band over o-blocks 0,1 while x
    # streams in.
    pe_seq = [("d", 24), ("xa", 0), ("d", 8), ("g", 0, 0), ("xa", 1),
              ("g", 1, 0), ("xa", 2),
              ("g", 0, 1), ("xa", 3), ("g", 2, 0), ("g", 1, 1), ("xa", 4),
              ("g", 3, 0), ("g", 2, 1), ("xa", 5), ("g", 4, 0), ("g", 3, 1),
              ("xa", 6), ("g", 5, 0), ("g", 4, 1), ("xa", 7), ("g", 6, 0),
              ("g", 5, 1), ("g", 7, 0), ("g", 6, 1), ("g", 7, 1)]
    for j in range(2, N_OBLK):
        for i in range(MT):
            pe_seq.append(("g", i, j))
    return dma_seq, pe_seq


def _build_nc(schedule=None):
    dma_seq, pe_seq = schedule if schedule is not None else _default_schedule()

    nc = bacc.Bacc(None, target_bir_lowering=False)

    xf8_d = nc.dram_tensor("xf8", [MT, P, KT, P], F8, kind="ExternalInput")
    xbf_d = nc.dram_tensor("xbf", [MT, P, KT, P], BF, kind="ExternalInput")
    wt_d = nc.dram_tensor("wt", [N_OBLK, P, KP, 2, O_BLK], F8,
                          kind="ExternalInput")
    at_d = nc.dram_tensor("at", [P, KT, RANK], BF, kind="ExternalInput")
    bb_d = nc.dram_tensor("bb", [RP, OUT_F], BF, kind="ExternalInput")
    id_d = nc.dram_tensor("ident", [P, P], BF, kind="ExternalInput")
    # [i][p][j][c] row-major == [1024, 4096] row-major
    out_d = nc.dram_tensor("out", [MT, P, N_OBLK, O_BLK], BF,
                           kind="ExternalOutput")

    with tile.TileContext(nc) as tc:
        with (
            tc.tile_pool(name="xp", bufs=1) as xp,
            tc.tile_pool(name="wp", bufs=3) as wp,
            tc.tile_pool(name="sm", bufs=1) as sm,
            tc.tile_pool(name="outs", bufs=2) as outs,
            tc.tile_pool(name="pmain", bufs=4, space="PSUM") as pmain,
            tc.tile_pool(name="pxa", bufs=2, space="PSUM") as pxa,
            tc.tile_pool(name="ptr", bufs=1, space="PSUM") as ptr,
        ):
            # ---- small constants ----
            at_sb = sm.tile([P, KT, RANK], BF, tag="at")
            nc.sync.dma_start(at_sb[:], at_d[:])
            bb_sb = sm.tile([RP, OUT_F], BF, tag="bb")
            nc.sync.dma_start(bb_sb[:], bb_d[:])
            id_sb = sm.tile([P, P], BF, tag="ident")
            nc.sync.dma_start(id_sb[:], id_d[:])
            xaT = sm.tile([RP, M_PER_CORE], BF, tag="xaT")
            nc.vector.memset(xaT[RANK:RP, :], 1.0)
            warm = sm.tile([P, P], BF, tag="warm")
            nc.vector.memset(warm[:], 0.0)
            wps = pxa.tile([P, O_BLK], mybir.dt.float32, tag="wps", bufs=1)

            xf8s, xbfs = [], []
            for i in range(MT):
                xf8s.append(xp.tile([P, KT, P], F8, tag=f"xf8_{i}",
                                    name=f"xf8_{i}"))
                xbfs.append(xp.tile([P, KT, P], BF, tag=f"xbf_{i}",
                                    name=f"xbf_{i}"))

            w_tiles = {}
            col_tiles = {}
            emitted_w = set()

            def dummies(n):
                for _ in range(n):
                    nc.tensor.matmul(wps[:, 0:P], warm[:], warm[:],
                                     start=True, stop=True)

            def dma_x(i):
                nc.sync.dma_start(xf8s[i][:], xf8_d[i])
                nc.sync.dma_start(xbfs[i][:], xbf_d[i])

            def dma_w(j):
                if j in emitted_w or j >= N_OBLK:
                    return
                emitted_w.add(j)
                t = wp.tile([P, KP, 2, O_BLK], F8, tag="w", name=f"w{j}")
                nc.sync.dma_start(t[:], wt_d[j])
                w_tiles[j] = t

            def xa(i):
                # xa_psum[tok, r] = sum_k x_bf16[tok, k] * A[r, k]
                ps = pxa.tile([P, RANK], mybir.dt.float32, tag="pxa",
                              name=f"pxa{i}")
                for k in range(KT):
                    nc.tensor.matmul(
                        ps[:], xbfs[i][:, k, :], at_sb[:, k, :],
                        start=(k == 0), stop=(k == KT - 1),
                    )
                xa_sb = sm.tile([P, RANK], BF, tag="xa_sb", bufs=2,
                                name=f"xa_sb{i}")
                nc.scalar.copy(xa_sb[:], ps[:])
                tr = ptr.tile([RANK, P], BF, tag="ptr", name=f"ptr{i}")
                nc.tensor.transpose(tr[:], xa_sb[:], id_sb[:])
                nc.scalar.copy(xaT[0:RANK, i * P:(i + 1) * P], tr[:])

            gidx = 0
            col_remaining = {j: MT for j in range(N_OBLK)}

            def group(i, j):
                nonlocal gidx
                if j not in col_tiles:
                    col_tiles[j] = outs.tile([P, MT, O_BLK], BF, tag="col",
                                             name=f"col{j}")
                ps = pmain.tile([P, O_BLK], mybir.dt.float32, tag="pm",
                                name=f"pm{i}_{j}")
                for kp in range(KP):
                    nc.tensor.matmul(
                        ps[:], xf8s[i][:, 2 * kp:2 * kp + 2, :],
                        w_tiles[j][:, kp, :, :],
                        start=(kp == 0), stop=False,
                        perf_mode=mybir.MatmulPerfMode.DoubleRow,
                    )
                nc.tensor.matmul(
                    ps[:], xaT[:, i * P:(i + 1) * P],
                    bb_sb[:, j * O_BLK:(j + 1) * O_BLK],
                    start=False, stop=True,
                )
                if gidx % 2 == 0:
                    nc.scalar.copy(col_tiles[j][:, i, :], ps[:])
                else:
                    nc.vector.tensor_copy(col_tiles[j][:, i, :], ps[:])
                gidx += 1
                col_remaining[j] -= 1
                if col_remaining[j] == MT // 2:
                    nc.sync.dma_start(
                        out_d[0:MT // 2, :, j, :].rearrange("i p c -> p i c"),
                        col_tiles[j][:, 0:MT // 2, :],
                    )
                if col_remaining[j] == 0:
                    nc.sync.dma_start(
                        out_d[MT // 2:MT, :, j, :].rearrange("i p c -> p i c"),
                        col_tiles[j][:, MT // 2:MT, :],
                    )
                    dma_w(j + 3)

            for tok in dma_seq:
                if tok[0] == "x":
                    dma_x(int(tok[1:]))
                else:
                    dma_w(int(tok[1:]))
            for op in pe_seq:
                if op[0] == "xa":
                    xa(op[1])
                elif op[0] == "d":
                    dummies(op[1])
                else:
                    group(op[1], op[2])
    nc.compile()
    return nc


def _prep_inputs(x, W, b, lora_A, lora_B):
    scaling = ALPHA / RANK
    Wq = (W.astype(np.float32) * SW).astype(NP_F8)            # [out, in]
    # wt[ob, p, kp, t, c] = Wq[ob*512 + c, (2*kp + t)*128 + p]
    wt_in = np.ascontiguousarray(
        Wq.T.reshape(KP, 2, P, N_OBLK, O_BLK).transpose(3, 2, 0, 1, 4)
    )

    at_in = np.ascontiguousarray(
        lora_A.astype(np.float32).T.reshape(KT, P, RANK).transpose(1, 0, 2)
    ).astype(NP_BF)                                           # [p, k, r]

    bb_in = np.zeros((RP, OUT_F), dtype=NP_BF)
    bb_in[0:RANK] = (lora_B.astype(np.float32).T * (scaling * SX * SW)).astype(NP_BF)
    bb_in[RANK] = (b.astype(np.float32) * (SX * SW)).astype(NP_BF)

    id_in = np.eye(P, dtype=NP_BF)

    x_flat = np.ascontiguousarray(x.reshape(TOK, IN_F).astype(np.float32))
    in_maps = []
    for c in range(N_CORES):
        xc = x_flat[c * M_PER_CORE:(c + 1) * M_PER_CORE]      # [1024, 4096]
        # x[m, p, k, c] = xc[m*128 + c, k*128 + p]
        xt = xc.T.reshape(KT, P, MT, P).transpose(2, 1, 0, 3)
        xf8_in = np.ascontiguousarray(xt * SX).astype(NP_F8)
        xbf_in = np.ascontiguousarray(xt).astype(NP_BF)
        in_maps.append({
            "xf8": xf8_in,
            "xbf": xbf_in,
            "wt": wt_in,
            "at": at_in,
            "bb": bb_in,
            "ident": id_in,
        })
    return in_maps


def kernel(x, W, b, lora_A, lora_B, _trace=False):
    global LAST_RESULTS, _NC_CACHE

    in_maps = _prep_inputs(x, W, b, lora_A, lora_B)

    if _NC_CACHE is None:
        _NC_CACHE = _build_nc()
    nc = _NC_CACHE

    res = run_bass_kernel_spmd(nc, in_maps, core_ids=list(range(N_CORES)),
                               trace=_trace)
    LAST_RESULTS = res

    out = np.concatenate(
        [r["out"].reshape(M_PER_CORE, OUT_F).astype(np.float32)
         for r in res.results], axis=0)
    out *= np.float32(DESCALE)
    return out.reshape(B_SZ, S_SZ, OUT_F).astype(np.float32)
